# revision 2
# baseline (speedup 1.0000x reference)
"""NT-Xent / SimCLR contrastive loss on 8 Trainium2 NeuronCores (Bass/Tile).

Problem: zi, zj [4096, 512] f32 -> scalar loss.
  reps = concat(zi, zj)            [8192, 512]
  rn   = reps / max(||reps||, 1e-8)
  sim  = rn @ rn.T                 [8192, 8192]
  pos_i  = sim[i, (i+B) mod 2B]
  denom_i = sum_{j != i} exp(sim_ij / tau)
  loss = mean(-pos/tau + log(denom))

Sharding: core c owns sim-matrix rows [c*1024, (c+1)*1024). Every core gets
the full reps (bf16), normalizes them redundantly on device (cheaper than an
all-gather at this size), builds the transposed GEMM operand via DMA-xbar
transposes (DRAM roundtrip), runs its row-block GEMM (fp8 DoubleRow by
default), and returns a [128, 16] tile of per-row denominators and
positives. The host applies log and the final mean.

Numerics:
- Diagonal exclusion is exact: the PE's own sim_ii is extracted from PSUM
  (identity-mask multiply+reduce, fp32), passed through the same ACT Exp
  LUT, and subtracted - the huge exp(sim_ii/tau) term cancels bit-exactly.
- Positives are computed separately as a bf16 partner-block dot (fp32
  accumulate), independent of the GEMM precision.
- Row norms: 1/sqrt(n2) via the bitcast-magic Newton iteration on the (idle)
  GpSimd engine - keeps the Scalar engine Exp-only (single ACT table load)
  and is accurate to ~1e-7 after two iterations.
"""

import sys

for _p in ("/opt/trn_rl_repo",):
    if _p not in sys.path:
        sys.path.insert(0, _p)

from contextlib import ExitStack

import ml_dtypes
import numpy as np

TAU = 0.07
B, D = 4096, 512
NCORES = 8
ROWS = 2 * B              # 8192
RPC = ROWS // NCORES      # 1024 rows per core
NM = RPC // 128           # 8 m-tiles per core
NFULL = ROWS // 128       # 64 natural 128-row tiles
NG = 4                    # norm/GEMM groups (2048 cols each)
GW = ROWS // NG           # 2048 group width
KC = D // 128             # 4 contraction chunks
NSEG = ROWS // 1024       # 8 transpose segments of 1024 rows

_prog_cache = {}

_cfg = {
    "fp8": True,    # fp8e4m3 DoubleRow GEMM (else bf16); ~5e-3 loss err
}


def _build_program():
    import concourse.bacc as bacc
    import concourse.tile as tile
    import concourse.mybir as mybir

    dt = mybir.dt
    Alu = mybir.AluOpType
    Act = mybir.ActivationFunctionType
    fp8 = _cfg["fp8"]
    gdt = dt.float8e4 if fp8 else dt.bfloat16

    nc = bacc.Bacc("TRN2", target_bir_lowering=False, debug=False,
                   enable_asserts=False, num_devices=NCORES)

    full_nat = nc.dram_tensor("full_nat", [NFULL, 128, D], dt.bfloat16,
                              kind="ExternalInput").ap()
    block_nat = nc.dram_tensor("block_nat", [NM, 128, D], dt.bfloat16,
                               kind="ExternalInput").ap()
    partner_nat = nc.dram_tensor("partner_nat", [NM, 128, D], dt.bfloat16,
                                 kind="ExternalInput").ap()
    ident_f32 = nc.dram_tensor("ident_f32", [128, 128], dt.float32,
                               kind="ExternalInput").ap()
    sdiag = nc.dram_tensor("sdiag", [128, NM * NG * 2], dt.float32,
                           kind="ExternalInput").ap()
    out = nc.dram_tensor("out", [128, 2 * NM], dt.float32,
                         kind="ExternalOutput").ap()

    with tile.TileContext(nc) as tc, ExitStack() as ctx:
        const = ctx.enter_context(tc.tile_pool(name="const", bufs=1))
        persist = ctx.enter_context(tc.tile_pool(name="persist", bufs=1))
        dramp = ctx.enter_context(tc.tile_pool(name="dramp", bufs=1,
                                               space="DRAM"))
        xin = ctx.enter_context(tc.tile_pool(name="xin", bufs=6))
        rnbufp = ctx.enter_context(tc.tile_pool(name="rnbufp", bufs=6))
        tseg = ctx.enter_context(tc.tile_pool(name="tseg", bufs=8))
        scrp = ctx.enter_context(tc.tile_pool(name="scrp", bufs=3))
        normp = ctx.enter_context(tc.tile_pool(name="normp", bufs=3))
        ep = ctx.enter_context(tc.tile_pool(name="ep", bufs=3))
        smallp = ctx.enter_context(tc.tile_pool(name="smallp", bufs=4))
        gpsum = ctx.enter_context(tc.tile_pool(name="gpsum", bufs=2,
                                               space="PSUM"))

        i32 = const.tile([128, 128], dt.float32, tag="i32")
        nc.sync.dma_start(i32[:], ident_f32[:])
        sd = const.tile([128, NM * NG * 2], dt.float32, tag="sd")
        nc.sync.dma_start(sd[:], sdiag[:])

        # resident GEMM operands (chunk c at columns [c*W, (c+1)*W))
        rnT = persist.tile([128, KC * ROWS], gdt, tag="rnT")
        lhsT = persist.tile([128, KC * RPC], gdt, tag="lhsT")
        outbuf = persist.tile([128, 2 * NM], dt.float32, tag="outbuf")
        dvtabs = persist.tile([128, NM * NG * 2], dt.float32, tag="dvtabs")
        rstabs = persist.tile([128, NM * NG], dt.float32, tag="rstabs")

        rnT_v = rnT[:].rearrange("p (c w) -> p c w", c=KC)
        lhsT_v = lhsT[:].rearrange("p (c w) -> p c w", c=KC)

        # DRAM scratch for normalized rows (transpose-DMA source)
        segs = [dramp.tile([8, 128, D], dt.bfloat16, tag=f"seg{s}",
                           name=f"seg{s}") for s in range(NSEG)]
        blockd = dramp.tile([NM, 128, D], dt.bfloat16, tag="blockd")

        def rsqrt_act(inv, n2):
            """inv = n2 ** -0.5 via Exp(-0.5*Ln(n2)) - both funcs live in the
            single natural_log_exp_and_others ACT table (no table thrash)."""
            lng = normp.tile([128, n2.shape[1]], dt.float32, tag="lng")
            nc.scalar.activation(lng[:], n2[:], Act.Ln)
            nc.scalar.activation(inv[:], lng[:], Act.Exp, scale=-0.5)

        def norm4(src4, store_to, sq_engine):
            """Load 4 natural 128-row tiles (one DMA), row-normalize to bf16,
            optionally store to DRAM scratch. Returns the [128, 4D] buffer."""
            x4 = xin.tile([128, 4 * D], dt.bfloat16, tag="x4")
            nc.sync.dma_start(x4[:].rearrange("p (a d) -> p a d", a=4),
                              src4.rearrange("a p d -> p a d"))
            n2 = normp.tile([128, 4], dt.float32, tag="n2")
            for k in range(4):
                scr = scrp.tile([128, D], dt.bfloat16, tag="scr512")
                sq_engine.scalar_tensor_tensor(
                    out=scr[:], in0=x4[:, k * D:(k + 1) * D], scalar=1.0,
                    in1=x4[:, k * D:(k + 1) * D], op0=Alu.mult, op1=Alu.mult,
                    accum_out=n2[:, k:k + 1])
            nc.vector.tensor_scalar(out=n2[:], in0=n2[:], scalar1=1e-16,
                                    scalar2=None, op0=Alu.max)
            inv = normp.tile([128, 4], dt.float32, tag="inv")
            rsqrt_act(inv, n2)
            rn4 = rnbufp.tile([128, 4 * D], dt.bfloat16, tag="rn4")
            for k in range(4):
                nc.vector.tensor_scalar_mul(rn4[:, k * D:(k + 1) * D],
                                            x4[:, k * D:(k + 1) * D],
                                            inv[:, k:k + 1])
            if store_to is not None:
                dst4, a0 = store_to
                nc.sync.dma_start(
                    dst4[a0:a0 + 4].rearrange("a p d -> p a d"),
                    rn4[:].rearrange("p (a d) -> p a d", a=4))
            return rn4

        cast_rr = [nc.scalar, nc.vector, nc.gpsimd]

        def transpose_in(src2d, dest_v, c, cols, cast_idx):
            """DMA-xbar transpose src2d rows -> dest_v[:, c, cols]. Split into
            512-row halves so each can start as soon as its rows are stored."""
            n = cols.stop - cols.start
            if not _cfg["fp8"]:
                for hh in range(0, n, 512):
                    nc.sync.dma_start_transpose(
                        dest_v[:, c, cols.start + hh:cols.start + hh + 512],
                        src2d[hh:hh + 512, c * 128:(c + 1) * 128])
                return
            st = tseg.tile([128, 1024], dt.bfloat16, tag="st")
            nc.sync.dma_start_transpose(st[:, :n],
                                        src2d[:, c * 128:(c + 1) * 128])
            eng = cast_rr[cast_idx % 3]
            if eng is nc.scalar:
                nc.scalar.copy(dest_v[:, c, cols], st[:, :n])
            else:
                eng.tensor_copy(dest_v[:, c, cols], st[:, :n])

        # ---- block rows first: lhsT is on the critical path ----
        bbufs = {}
        blk2d = blockd[:].rearrange("a p d -> (a p) d")
        for a in range(0, NM, 4):
            bbufs[a] = norm4(block_nat[a:a + 4], (blockd, a), nc.vector)
        for c in range(KC):
            transpose_in(blk2d, lhsT_v, c, slice(0, RPC), c)

        inv_tau = float(1.0 / TAU)
        ncast = KC
        for g in range(NG):
            for half in range(2):
                s = 2 * g + half
                for i, a in enumerate((0, 4)):
                    norm4(full_nat[s * 8 + a:s * 8 + a + 4], (segs[s], a),
                          nc.vector)
                s2d = segs[s][:].rearrange("a p d -> (a p) d")
                for c in range(KC):
                    transpose_in(s2d, rnT_v, c,
                                 slice(s * 1024, (s + 1) * 1024), ncast)
                    ncast += 1
            # GEMM over this 2048-column block
            for t in range(NM):
                ps = gpsum.tile([128, GW], dt.float32, tag="ps")
                for h in range(4):
                    if _cfg["fp8"]:
                        for cp in range(2):
                            nc.tensor.matmul(
                                ps[:, h * 512:(h + 1) * 512],
                                lhsT_v[:, 2 * cp:2 * cp + 2,
                                       t * 128:(t + 1) * 128],
                                rnT_v[:, 2 * cp:2 * cp + 2,
                                      g * GW + h * 512:g * GW + (h + 1) * 512],
                                perf_mode=mybir.MatmulPerfMode.DoubleRow,
                                start=(cp == 0), stop=(cp == 1))
                    else:
                        for c in range(KC):
                            nc.tensor.matmul(
                                ps[:, h * 512:(h + 1) * 512],
                                lhsT_v[:, c, t * 128:(t + 1) * 128],
                                rnT_v[:, c, g * GW + h * 512:
                                      g * GW + (h + 1) * 512],
                                start=(c == 0), stop=(c == KC - 1))
                # exact diagonal candidates, one per 1024-half
                for half in range(2):
                    o = half * 1024 + t * 128
                    scr = scrp.tile([128, 128], dt.float32, tag="scr128")
                    col = t * NG * 2 + g * 2 + half
                    nc.vector.scalar_tensor_tensor(
                        out=scr[:], in0=ps[:, o:o + 128], scalar=1.0,
                        in1=i32[:], op0=Alu.mult, op1=Alu.mult,
                        accum_out=dvtabs[:, col:col + 1])
                e = ep.tile([128, GW], dt.float32, tag="e")
                nc.scalar.activation(
                    e[:], ps[:], Act.Exp, scale=inv_tau,
                    accum_out=rstabs[:, t * NG + g:t * NG + g + 1])

        # ---- positives (off the critical path) ----
        for a in range(0, NM, 4):
            prn = norm4(partner_nat[a:a + 4], None, nc.vector)
            brn = bbufs[a]
            for k in range(4):
                t = a + k
                scr = scrp.tile([128, D], dt.bfloat16, tag="scr512")
                nc.vector.scalar_tensor_tensor(
                    out=scr[:], in0=brn[:, k * D:(k + 1) * D], scalar=1.0,
                    in1=prn[:, k * D:(k + 1) * D], op0=Alu.mult, op1=Alu.mult,
                    accum_out=outbuf[:, NM + t:NM + t + 1])

        # ---- epilogue per m-tile ----
        for t in range(NM):
            w = NG * 2
            scr16 = smallp.tile([128, w], dt.float32, tag="scr16")
            selfsim = smallp.tile([128, 1], dt.float32, tag="selfsim")
            nc.vector.scalar_tensor_tensor(
                out=scr16[:], in0=dvtabs[:, t * w:(t + 1) * w], scalar=1.0,
                in1=sd[:, t * w:(t + 1) * w], op0=Alu.mult, op1=Alu.mult,
                accum_out=selfsim[:])
            selfexp = smallp.tile([128, 1], dt.float32, tag="selfexp")
            nc.scalar.activation(selfexp[:], selfsim[:], Act.Exp,
                                 scale=inv_tau)
            rowsum = smallp.tile([128, 1], dt.float32, tag="rowsum")
            nc.vector.reduce_sum(rowsum[:], rstabs[:, t * NG:(t + 1) * NG],
                                 axis=mybir.AxisListType.X)
            nc.vector.tensor_sub(outbuf[:, t:t + 1], rowsum[:], selfexp[:])

        nc.sync.dma_start(out[:], outbuf[:])

    # Restrict bacc's activation-table choices to the one table that holds
    # Ln+Exp+Copy together, so exactly one ACT table load is emitted (the
    # default greedy choice alternates tables and costs ~1.3us per switch).
    import concourse.bacc as bacc_mod
    _orig_tables = bacc_mod.get_activation_tables

    def _only_lnexp(arch):
        keep = "natural_log_exp_and_others"
        return {k: (v if k == keep else set())
                for k, v in _orig_tables(arch).items()}

    bacc_mod.get_activation_tables = _only_lnexp
    try:
        nc.compile()
    finally:
        bacc_mod.get_activation_tables = _orig_tables
    return nc


def _host_inputs(zi, zj):
    reps = np.concatenate([np.asarray(zi, np.float32),
                           np.asarray(zj, np.float32)], axis=0)
    reps_bf = reps.astype(ml_dtypes.bfloat16)
    full_nat = np.ascontiguousarray(reps_bf.reshape(NFULL, 128, D))
    ident_f32 = np.eye(128, dtype=np.float32)
    in_maps = []
    for c in range(NCORES):
        pc = (c + NCORES // 2) % NCORES
        block = np.ascontiguousarray(
            reps_bf[c * RPC:(c + 1) * RPC].reshape(NM, 128, D))
        partner = np.ascontiguousarray(
            reps_bf[pc * RPC:(pc + 1) * RPC].reshape(NM, 128, D))
        # diag candidate selector: for m-tile t, candidate (g, half) is the
        # real diagonal iff its 1024-block index (2g+half) == c
        sdv = np.zeros((128, NM * NG * 2), np.float32)
        for t in range(NM):
            sdv[:, t * NG * 2 + c] = 1.0
        in_maps.append({
            "full_nat": full_nat, "block_nat": block, "partner_nat": partner,
            "ident_f32": ident_f32, "sdiag": sdv,
        })
    return in_maps


def _postprocess(results):
    denom = np.empty((ROWS,), np.float64)
    pos = np.empty((ROWS,), np.float64)
    for c in range(NCORES):
        o = np.asarray(results[c]["out"], np.float64)  # [128, 16]
        for t in range(NM):
            rows = slice(c * RPC + t * 128, c * RPC + (t + 1) * 128)
            denom[rows] = o[:, t]
            pos[rows] = o[:, NM + t]
    loss = np.mean(-pos / TAU + np.log(denom))
    return np.asarray(loss, dtype=np.float32)


def kernel(zi, zj, _trace=False):
    from concourse.bass_utils import run_bass_kernel_spmd

    if "nc" not in _prog_cache:
        _prog_cache["nc"] = _build_program()
    nc = _prog_cache["nc"]
    in_maps = _host_inputs(zi, zj)
    res = run_bass_kernel_spmd(nc, in_maps, list(range(NCORES)),
                               trace=_trace)
    _prog_cache["last_result"] = res
    return _postprocess(res.results)



# revision 3
# speedup vs baseline: 1.8787x; 1.8787x over previous
"""NT-Xent / SimCLR contrastive loss on 8 Trainium2 NeuronCores (Bass/Tile).

Problem: zi, zj [4096, 512] f32 -> scalar loss.
  reps = concat(zi, zj)            [8192, 512]
  rn   = reps / max(||reps||, 1e-8)
  sim  = rn @ rn.T                 [8192, 8192]
  pos_i  = sim[i, (i+B) mod 2B]
  denom_i = sum_{j != i} exp(sim_ij / tau)
  loss = mean(-pos/tau + log(denom))

Sharding (per the hint: each device holds its row block of normalized reps
plus the full normalized reps for the GEMM): core c owns sim rows
[c*1024, (c+1)*1024). The host normalizes in f32 (identical math to the
reference) and ships the full normalized reps transposed + quantized to
fp8e4m3 (rnT, replicated) and the core's own row-block slice (lhsT). The
device then runs a pure fp8 DoubleRow GEMM -> Exp -> row-reduce pipeline
with nothing on the critical path ahead of the first matmul.

Numerics:
- Diagonal exclusion is exact: sim_ii is extracted from PSUM (identity-mask
  multiply+reduce, f32), passed through the same ACT Exp LUT, and
  subtracted, so the huge exp(sim_ii/tau) term cancels bit-exactly.
- Positives come from the same PSUM via the partner-block diagonal. For
  m-tile t and 2048-col group g, the diagonal of local 128-col blocks t and
  t+8 covers, across g, all 8 possible positions of both the self and the
  partner diagonals (position 2g+half == c resp. (c+4) mod 8); host-side
  one-hot masks select the right candidate per core.
"""

import sys

for _p in ("/opt/trn_rl_repo",):
    if _p not in sys.path:
        sys.path.insert(0, _p)

from contextlib import ExitStack

import ml_dtypes
import numpy as np

TAU = 0.07
B, D = 4096, 512
NCORES = 8
ROWS = 2 * B              # 8192
RPC = ROWS // NCORES      # 1024 rows per core
NM = RPC // 128           # 8 m-tiles per core
KC = D // 128             # 4 contraction chunks
NG = 4                    # column groups
GW = ROWS // NG           # 2048 cols per group
NCAND = 2 * NG            # 8 diag candidates per m-tile

_prog_cache = {}


def _build_program():
    import concourse.bacc as bacc
    import concourse.tile as tile
    import concourse.mybir as mybir

    dt = mybir.dt
    Alu = mybir.AluOpType
    Act = mybir.ActivationFunctionType

    nc = bacc.Bacc("TRN2", target_bir_lowering=False, debug=False,
                   enable_asserts=False, num_devices=NCORES)

    rnT_in = nc.dram_tensor("rnT", [KC, 128, ROWS], dt.float8e4,
                            kind="ExternalInput").ap()
    lhsT_in = nc.dram_tensor("lhsT", [KC, 128, RPC], dt.float8e4,
                             kind="ExternalInput").ap()
    ident_in = nc.dram_tensor("ident_f32", [128, 128], dt.float32,
                              kind="ExternalInput").ap()
    smask_in = nc.dram_tensor("selfmask", [128, NCAND], dt.float32,
                              kind="ExternalInput").ap()
    pmask_in = nc.dram_tensor("posmask", [128, NCAND], dt.float32,
                              kind="ExternalInput").ap()
    out = nc.dram_tensor("out", [128, 2 * NM], dt.float32,
                         kind="ExternalOutput").ap()

    inv_tau = float(1.0 / TAU)

    with tile.TileContext(nc) as tc, ExitStack() as ctx:
        const = ctx.enter_context(tc.tile_pool(name="const", bufs=1))
        persist = ctx.enter_context(tc.tile_pool(name="persist", bufs=1))
        ep = ctx.enter_context(tc.tile_pool(name="ep", bufs=3))
        scrp = ctx.enter_context(tc.tile_pool(name="scrp", bufs=4))
        smallp = ctx.enter_context(tc.tile_pool(name="smallp", bufs=4))
        gpsum = ctx.enter_context(tc.tile_pool(name="gpsum", bufs=2,
                                               space="PSUM"))

        i32 = const.tile([128, 128], dt.float32, tag="i32")
        nc.sync.dma_start(i32[:], ident_in[:])
        smask = const.tile([128, NCAND], dt.float32, tag="smask")
        nc.sync.dma_start(smask[:], smask_in[:])
        pmask = const.tile([128, NCAND], dt.float32, tag="pmask")
        nc.sync.dma_start(pmask[:], pmask_in[:])

        lhsT = persist.tile([128, KC * RPC], dt.float8e4, tag="lhsT")
        lhsT_v = lhsT[:].rearrange("p (c w) -> p c w", c=KC)
        for c in range(KC):
            nc.sync.dma_start(lhsT_v[:, c, :], lhsT_in[c])

        rnT = persist.tile([128, KC * ROWS], dt.float8e4, tag="rnT")
        rnT_v = rnT[:].rearrange("p (c w) -> p c w", c=KC)
        # group-major arrival order so GEMM on group 0 starts immediately
        for g in range(NG):
            for c in range(KC):
                nc.sync.dma_start(rnT_v[:, c, g * GW:(g + 1) * GW],
                                  rnT_in[c, :, g * GW:(g + 1) * GW])

        dvtabs = persist.tile([128, NM * NCAND], dt.float32, tag="dvtabs")
        rstabs = persist.tile([128, NM * NG], dt.float32, tag="rstabs")
        outbuf = persist.tile([128, 2 * NM], dt.float32, tag="outbuf")

        for g in range(NG):
            for t in range(NM):
                ps = gpsum.tile([128, GW], dt.float32, tag="ps")
                for cp in range(2):
                    for h in range(4):
                        nc.tensor.matmul(
                            ps[:, h * 512:(h + 1) * 512],
                            lhsT_v[:, 2 * cp:2 * cp + 2,
                                   t * 128:(t + 1) * 128],
                            rnT_v[:, 2 * cp:2 * cp + 2,
                                  g * GW + h * 512:g * GW + (h + 1) * 512],
                            perf_mode=mybir.MatmulPerfMode.DoubleRow,
                            start=(cp == 0), stop=(cp == 1))
                # diag candidates: local blocks t and t+8 (self or partner
                # diagonal when 2g+half == c resp. (c+4)%8)
                for half in range(2):
                    o = (t + 8 * half) * 128
                    scr = scrp.tile([128, 128], dt.float32, tag="scr128")
                    col = t * NCAND + 2 * g + half
                    nc.vector.scalar_tensor_tensor(
                        out=scr[:], in0=ps[:, o:o + 128], scalar=1.0,
                        in1=i32[:], op0=Alu.mult, op1=Alu.mult,
                        accum_out=dvtabs[:, col:col + 1])
                e = ep.tile([128, GW], dt.bfloat16, tag="e")
                nc.scalar.activation(
                    e[:], ps[:], Act.Exp, scale=inv_tau,
                    accum_out=rstabs[:, t * NG + g:t * NG + g + 1])

        # ---- epilogue per m-tile ----
        for t in range(NM):
            scr8 = smallp.tile([128, NCAND], dt.float32, tag="scr8")
            selfsim = smallp.tile([128, 1], dt.float32, tag="selfsim")
            nc.vector.scalar_tensor_tensor(
                out=scr8[:], in0=dvtabs[:, t * NCAND:(t + 1) * NCAND],
                scalar=1.0, in1=smask[:], op0=Alu.mult, op1=Alu.mult,
                accum_out=selfsim[:])
            scr8b = smallp.tile([128, NCAND], dt.float32, tag="scr8b")
            nc.vector.scalar_tensor_tensor(
                out=scr8b[:], in0=dvtabs[:, t * NCAND:(t + 1) * NCAND],
                scalar=1.0, in1=pmask[:], op0=Alu.mult, op1=Alu.mult,
                accum_out=outbuf[:, NM + t:NM + t + 1])
            selfexp = smallp.tile([128, 1], dt.float32, tag="selfexp")
            nc.scalar.activation(selfexp[:], selfsim[:], Act.Exp,
                                 scale=inv_tau)
            rowsum = smallp.tile([128, 1], dt.float32, tag="rowsum")
            nc.vector.reduce_sum(rowsum[:], rstabs[:, t * NG:(t + 1) * NG],
                                 axis=mybir.AxisListType.X)
            nc.vector.tensor_sub(outbuf[:, t:t + 1], rowsum[:], selfexp[:])

        nc.sync.dma_start(out[:], outbuf[:])

    # Restrict bacc's activation-table choices to the one table that holds
    # Exp+Copy together, so exactly one ACT table load is emitted.
    import concourse.bacc as bacc_mod
    _orig_tables = bacc_mod.get_activation_tables

    def _only_lnexp(arch):
        keep = "natural_log_exp_and_others"
        return {k: (v if k == keep else set())
                for k, v in _orig_tables(arch).items()}

    bacc_mod.get_activation_tables = _only_lnexp
    try:
        nc.compile()
    finally:
        bacc_mod.get_activation_tables = _orig_tables
    return nc


def _host_inputs(zi, zj):
    reps = np.concatenate([np.asarray(zi, np.float32),
                           np.asarray(zj, np.float32)], axis=0)
    norms = np.maximum(np.sqrt((reps * reps).sum(axis=1, keepdims=True)),
                       1e-8)
    rn8 = (reps / norms).astype(ml_dtypes.float8_e4m3fn)        # [2B, D]
    rnT = np.ascontiguousarray(rn8.T.reshape(KC, 128, ROWS))
    ident_f32 = np.eye(128, dtype=np.float32)
    in_maps = []
    for c in range(NCORES):
        lhsT = np.ascontiguousarray(
            rn8[c * RPC:(c + 1) * RPC].T.reshape(KC, 128, RPC))
        smask = np.zeros((128, NCAND), np.float32)
        smask[:, c] = 1.0
        pmask = np.zeros((128, NCAND), np.float32)
        pmask[:, (c + 4) % 8] = 1.0
        in_maps.append({
            "rnT": rnT, "lhsT": lhsT, "ident_f32": ident_f32,
            "selfmask": smask, "posmask": pmask,
        })
    return in_maps


def _postprocess(results):
    denom = np.empty((ROWS,), np.float64)
    pos = np.empty((ROWS,), np.float64)
    for c in range(NCORES):
        o = np.asarray(results[c]["out"], np.float64)  # [128, 16]
        for t in range(NM):
            rows = slice(c * RPC + t * 128, c * RPC + (t + 1) * 128)
            denom[rows] = o[:, t]
            pos[rows] = o[:, NM + t]
    loss = np.mean(-pos / TAU + np.log(denom))
    return np.asarray(loss, dtype=np.float32)


def kernel(zi, zj, _trace=False):
    from concourse.bass_utils import run_bass_kernel_spmd

    if "nc" not in _prog_cache:
        _prog_cache["nc"] = _build_program()
    nc = _prog_cache["nc"]
    in_maps = _host_inputs(zi, zj)
    res = run_bass_kernel_spmd(nc, in_maps, list(range(NCORES)),
                               trace=_trace)
    _prog_cache["last_result"] = res
    return _postprocess(res.results)


# revision 5
# speedup vs baseline: 1.9151x; 1.0194x over previous
"""NT-Xent / SimCLR contrastive loss on 8 Trainium2 NeuronCores (Bass/Tile).

Problem: zi, zj [4096, 512] f32 -> scalar loss.
  reps = concat(zi, zj)            [8192, 512]
  rn   = reps / max(||reps||, 1e-8)
  sim  = rn @ rn.T                 [8192, 8192]
  pos_i  = sim[i, (i+B) mod 2B]
  denom_i = sum_{j != i} exp(sim_ij / tau)
  loss = mean(-pos/tau + log(denom))

Sharding (per the hint: each device holds its row block of normalized reps
plus the full normalized reps for the GEMM): core c owns sim rows
[c*1024, (c+1)*1024). The host normalizes in f32 (identical math to the
reference) and ships the full normalized reps transposed + quantized to
fp8e4m3 (rnT, replicated) and the core's own row-block slice (lhsT). The
device then runs a pure fp8 DoubleRow GEMM -> Exp -> row-reduce pipeline
with nothing on the critical path ahead of the first matmul.

Numerics:
- Diagonal exclusion is exact: sim_ii is extracted from PSUM (identity-mask
  multiply+reduce, f32), passed through the same ACT Exp LUT, and
  subtracted, so the huge exp(sim_ii/tau) term cancels bit-exactly.
- Positives come from the same PSUM via the partner-block diagonal. For
  m-tile t and 2048-col group g, the diagonal of local 128-col blocks t and
  t+8 covers, across g, all 8 possible positions of both the self and the
  partner diagonals (position 2g+half == c resp. (c+4) mod 8); host-side
  one-hot masks select the right candidate per core.
"""

import sys

for _p in ("/opt/trn_rl_repo",):
    if _p not in sys.path:
        sys.path.insert(0, _p)

from contextlib import ExitStack

import ml_dtypes
import numpy as np

TAU = 0.07
B, D = 4096, 512
NCORES = 8
ROWS = 2 * B              # 8192
RPC = ROWS // NCORES      # 1024 rows per core
NM = RPC // 128           # 8 m-tiles per core
KC = D // 128             # 4 contraction chunks
NG = 4                    # column groups
GW = ROWS // NG           # 2048 cols per group
NCAND = 2 * NG            # 8 diag candidates per m-tile

_prog_cache = {}


def _build_program():
    import concourse.bacc as bacc
    import concourse.tile as tile
    import concourse.mybir as mybir

    dt = mybir.dt
    Alu = mybir.AluOpType
    Act = mybir.ActivationFunctionType

    nc = bacc.Bacc("TRN2", target_bir_lowering=False, debug=False,
                   enable_asserts=False, num_devices=NCORES)

    rnT_in = nc.dram_tensor("rnT", [KC, 128, ROWS], dt.float8e4,
                            kind="ExternalInput").ap()
    lhsT_in = nc.dram_tensor("lhsT", [KC, 128, RPC], dt.float8e4,
                             kind="ExternalInput").ap()
    ident_in = nc.dram_tensor("ident_f32", [128, 128], dt.float32,
                              kind="ExternalInput").ap()
    smask_in = nc.dram_tensor("selfmask", [128, NCAND], dt.float32,
                              kind="ExternalInput").ap()
    pmask_in = nc.dram_tensor("posmask", [128, NCAND], dt.float32,
                              kind="ExternalInput").ap()
    out = nc.dram_tensor("out", [128, 2 * NM], dt.float32,
                         kind="ExternalOutput").ap()

    inv_tau = float(1.0 / TAU)

    with tile.TileContext(nc) as tc, ExitStack() as ctx:
        const = ctx.enter_context(tc.tile_pool(name="const", bufs=1))
        persist = ctx.enter_context(tc.tile_pool(name="persist", bufs=1))
        ep = ctx.enter_context(tc.tile_pool(name="ep", bufs=3))
        scrp = ctx.enter_context(tc.tile_pool(name="scrp", bufs=4))
        smallp = ctx.enter_context(tc.tile_pool(name="smallp", bufs=4))
        gpsum = ctx.enter_context(tc.tile_pool(name="gpsum", bufs=2,
                                               space="PSUM"))

        i32 = const.tile([128, 128], dt.float32, tag="i32")
        nc.sync.dma_start(i32[:], ident_in[:])
        smask = const.tile([128, NCAND], dt.float32, tag="smask")
        nc.sync.dma_start(smask[:], smask_in[:])
        pmask = const.tile([128, NCAND], dt.float32, tag="pmask")
        nc.sync.dma_start(pmask[:], pmask_in[:])

        lhsT = persist.tile([128, KC * RPC], dt.float8e4, tag="lhsT")
        lhsT_v = lhsT[:].rearrange("p (c w) -> p c w", c=KC)
        rnT = persist.tile([128, KC * ROWS], dt.float8e4, tag="rnT")
        rnT_v = rnT[:].rearrange("p (c w) -> p c w", c=KC)

        # Spread input DMAs across the five engine-dynamic queues so the
        # first GEMM unit's eight dependencies (lhsT + rnT group 0) land in
        # parallel instead of serializing on the sync queue.
        qs = [nc.sync, nc.scalar, nc.gpsimd]
        nq = 0

        def qdma(dst, src):
            nonlocal nq
            qs[nq % len(qs)].dma_start(dst, src)
            nq += 1

        for c in range(KC):
            qdma(lhsT_v[:, c, :], lhsT_in[c])
        # group-major arrival order so GEMM on group 0 starts immediately
        for g in range(NG):
            for c in range(KC):
                qdma(rnT_v[:, c, g * GW:(g + 1) * GW],
                     rnT_in[c, :, g * GW:(g + 1) * GW])

        dvtabs = persist.tile([128, NM * NCAND], dt.float32, tag="dvtabs")
        rstabs = persist.tile([128, NM * NG], dt.float32, tag="rstabs")
        outbuf = persist.tile([128, 2 * NM], dt.float32, tag="outbuf")

        for g in range(NG):
            for t in range(NM):
                ps = gpsum.tile([128, GW], dt.float32, tag="ps")
                for cp in range(2):
                    for h in range(4):
                        nc.tensor.matmul(
                            ps[:, h * 512:(h + 1) * 512],
                            lhsT_v[:, 2 * cp:2 * cp + 2,
                                   t * 128:(t + 1) * 128],
                            rnT_v[:, 2 * cp:2 * cp + 2,
                                  g * GW + h * 512:g * GW + (h + 1) * 512],
                            perf_mode=mybir.MatmulPerfMode.DoubleRow,
                            start=(cp == 0), stop=(cp == 1))
                # diag candidates: local blocks t and t+8 (self or partner
                # diagonal when 2g+half == c resp. (c+4)%8)
                for half in range(2):
                    o = (t + 8 * half) * 128
                    scr = scrp.tile([128, 128], dt.float32, tag="scr128")
                    col = t * NCAND + 2 * g + half
                    nc.vector.scalar_tensor_tensor(
                        out=scr[:], in0=ps[:, o:o + 128], scalar=1.0,
                        in1=i32[:], op0=Alu.mult, op1=Alu.mult,
                        accum_out=dvtabs[:, col:col + 1])
                e = ep.tile([128, GW], dt.bfloat16, tag="e")
                nc.scalar.activation(
                    e[:], ps[:], Act.Exp, scale=inv_tau,
                    accum_out=rstabs[:, t * NG + g:t * NG + g + 1])

        # ---- epilogue per m-tile ----
        for t in range(NM):
            scr8 = smallp.tile([128, NCAND], dt.float32, tag="scr8")
            selfsim = smallp.tile([128, 1], dt.float32, tag="selfsim")
            nc.vector.scalar_tensor_tensor(
                out=scr8[:], in0=dvtabs[:, t * NCAND:(t + 1) * NCAND],
                scalar=1.0, in1=smask[:], op0=Alu.mult, op1=Alu.mult,
                accum_out=selfsim[:])
            scr8b = smallp.tile([128, NCAND], dt.float32, tag="scr8b")
            nc.vector.scalar_tensor_tensor(
                out=scr8b[:], in0=dvtabs[:, t * NCAND:(t + 1) * NCAND],
                scalar=1.0, in1=pmask[:], op0=Alu.mult, op1=Alu.mult,
                accum_out=outbuf[:, NM + t:NM + t + 1])
            selfexp = smallp.tile([128, 1], dt.float32, tag="selfexp")
            nc.scalar.activation(selfexp[:], selfsim[:], Act.Exp,
                                 scale=inv_tau)
            rowsum = smallp.tile([128, 1], dt.float32, tag="rowsum")
            nc.vector.reduce_sum(rowsum[:], rstabs[:, t * NG:(t + 1) * NG],
                                 axis=mybir.AxisListType.X)
            nc.vector.tensor_sub(outbuf[:, t:t + 1], rowsum[:], selfexp[:])

        nc.sync.dma_start(out[:], outbuf[:])

    # Restrict bacc's activation-table choices to the one table that holds
    # Exp+Copy together, so exactly one ACT table load is emitted.
    import concourse.bacc as bacc_mod
    _orig_tables = bacc_mod.get_activation_tables

    def _only_lnexp(arch):
        keep = "natural_log_exp_and_others"
        return {k: (v if k == keep else set())
                for k, v in _orig_tables(arch).items()}

    bacc_mod.get_activation_tables = _only_lnexp
    try:
        nc.compile()
    finally:
        bacc_mod.get_activation_tables = _orig_tables
    return nc


def _host_inputs(zi, zj):
    reps = np.concatenate([np.asarray(zi, np.float32),
                           np.asarray(zj, np.float32)], axis=0)
    norms = np.maximum(np.sqrt((reps * reps).sum(axis=1, keepdims=True)),
                       1e-8)
    rn8 = (reps / norms).astype(ml_dtypes.float8_e4m3fn)        # [2B, D]
    rnT = np.ascontiguousarray(rn8.T.reshape(KC, 128, ROWS))
    ident_f32 = np.eye(128, dtype=np.float32)
    in_maps = []
    for c in range(NCORES):
        lhsT = np.ascontiguousarray(
            rn8[c * RPC:(c + 1) * RPC].T.reshape(KC, 128, RPC))
        smask = np.zeros((128, NCAND), np.float32)
        smask[:, c] = 1.0
        pmask = np.zeros((128, NCAND), np.float32)
        pmask[:, (c + 4) % 8] = 1.0
        in_maps.append({
            "rnT": rnT, "lhsT": lhsT, "ident_f32": ident_f32,
            "selfmask": smask, "posmask": pmask,
        })
    return in_maps


def _postprocess(results):
    denom = np.empty((ROWS,), np.float64)
    pos = np.empty((ROWS,), np.float64)
    for c in range(NCORES):
        o = np.asarray(results[c]["out"], np.float64)  # [128, 16]
        for t in range(NM):
            rows = slice(c * RPC + t * 128, c * RPC + (t + 1) * 128)
            denom[rows] = o[:, t]
            pos[rows] = o[:, NM + t]
    loss = np.mean(-pos / TAU + np.log(denom))
    return np.asarray(loss, dtype=np.float32)


def kernel(zi, zj, _trace=False):
    from concourse.bass_utils import run_bass_kernel_spmd

    if "nc" not in _prog_cache:
        _prog_cache["nc"] = _build_program()
    nc = _prog_cache["nc"]
    in_maps = _host_inputs(zi, zj)
    res = run_bass_kernel_spmd(nc, in_maps, list(range(NCORES)),
                               trace=_trace)
    _prog_cache["last_result"] = res
    return _postprocess(res.results)


# revision 6
# speedup vs baseline: 1.9676x; 1.0274x over previous
"""NT-Xent / SimCLR contrastive loss on 8 Trainium2 NeuronCores (Bass/Tile).

Problem: zi, zj [4096, 512] f32 -> scalar loss.
  reps = concat(zi, zj)            [8192, 512]
  rn   = reps / max(||reps||, 1e-8)
  sim  = rn @ rn.T                 [8192, 8192]
  pos_i  = sim[i, (i+B) mod 2B]
  denom_i = sum_{j != i} exp(sim_ij / tau)
  loss = mean(-pos/tau + log(denom))

Sharding (per the hint: each device holds its row block of normalized reps
plus the full normalized reps for the GEMM): core c owns sim rows
[c*1024, (c+1)*1024). The host normalizes in f32 (identical math to the
reference) and ships the full normalized reps transposed + quantized to
fp8e4m3 (rnT, replicated) and the core's own row-block slice (lhsT). The
device then runs a pure fp8 DoubleRow GEMM -> Exp -> row-reduce pipeline
with nothing on the critical path ahead of the first matmul.

Numerics:
- Diagonal exclusion is exact: sim_ii is extracted from PSUM (identity-mask
  multiply+reduce, f32), passed through the same ACT Exp LUT, and
  subtracted, so the huge exp(sim_ii/tau) term cancels bit-exactly.
- Positives come from the same PSUM via the partner-block diagonal. For
  m-tile t and 2048-col group g, the diagonal of local 128-col blocks t and
  t+8 covers, across g, all 8 possible positions of both the self and the
  partner diagonals (position 2g+half == c resp. (c+4) mod 8); host-side
  one-hot masks select the right candidate per core.
"""

import sys

for _p in ("/opt/trn_rl_repo",):
    if _p not in sys.path:
        sys.path.insert(0, _p)

from contextlib import ExitStack

import ml_dtypes
import numpy as np

TAU = 0.07
B, D = 4096, 512
NCORES = 8
ROWS = 2 * B              # 8192
RPC = ROWS // NCORES      # 1024 rows per core
NM = RPC // 128           # 8 m-tiles per core
KC = D // 128             # 4 contraction chunks
NG = 4                    # column groups
GW = ROWS // NG           # 2048 cols per group
NCAND = 2 * NG            # 8 diag candidates per m-tile

_prog_cache = {}


def _build_program():
    import concourse.bacc as bacc
    import concourse.tile as tile
    import concourse.mybir as mybir

    dt = mybir.dt
    Alu = mybir.AluOpType
    Act = mybir.ActivationFunctionType

    nc = bacc.Bacc("TRN2", target_bir_lowering=False, debug=False,
                   enable_asserts=False, num_devices=NCORES)

    rnT_in = nc.dram_tensor("rnT", [KC, 128, ROWS], dt.float8e4,
                            kind="ExternalInput").ap()
    lhsT_in = nc.dram_tensor("lhsT", [KC, 128, RPC], dt.float8e4,
                             kind="ExternalInput").ap()
    ident_in = nc.dram_tensor("ident_f32", [128, 128], dt.float32,
                              kind="ExternalInput").ap()
    smask_in = nc.dram_tensor("selfmask", [128, NCAND], dt.float32,
                              kind="ExternalInput").ap()
    pmask_in = nc.dram_tensor("posmask", [128, NCAND], dt.float32,
                              kind="ExternalInput").ap()
    out = nc.dram_tensor("out", [128, 2 * NM], dt.float32,
                         kind="ExternalOutput").ap()

    inv_tau = float(1.0 / TAU)

    with tile.TileContext(nc) as tc, ExitStack() as ctx:
        const = ctx.enter_context(tc.tile_pool(name="const", bufs=1))
        persist = ctx.enter_context(tc.tile_pool(name="persist", bufs=1))
        ep = ctx.enter_context(tc.tile_pool(name="ep", bufs=3))
        scrp = ctx.enter_context(tc.tile_pool(name="scrp", bufs=4))
        smallp = ctx.enter_context(tc.tile_pool(name="smallp", bufs=4))
        gpsum = ctx.enter_context(tc.tile_pool(name="gpsum", bufs=2,
                                               space="PSUM"))

        i32 = const.tile([128, 128], dt.float32, tag="i32")
        smask = const.tile([128, NCAND], dt.float32, tag="smask")
        pmask = const.tile([128, NCAND], dt.float32, tag="pmask")

        lhsT = persist.tile([128, KC * RPC], dt.float8e4, tag="lhsT")
        lhsT_v = lhsT[:].rearrange("p (c w) -> p c w", c=KC)
        rnT = persist.tile([128, KC * ROWS], dt.float8e4, tag="rnT")
        rnT_v = rnT[:].rearrange("p (c w) -> p c w", c=KC)

        # Input DMAs ride the sync + gpsimd queues only (both engines are
        # otherwise idle; issuing from scalar would delay the first Exp since
        # engine instruction streams are in-order). The first GEMM unit's
        # dependencies (rnT group 0 chunks 0-1, lhsT chunks 0-1) are issued
        # first so the PE starts as early as possible.
        qs = [nc.sync, nc.gpsimd]
        nq = 0

        def qdma(dst, src):
            nonlocal nq
            qs[nq % len(qs)].dma_start(dst, src)
            nq += 1

        def load_rnT(g, c):
            qdma(rnT_v[:, c, g * GW:(g + 1) * GW],
                 rnT_in[c, :, g * GW:(g + 1) * GW])

        def load_lhsT(c):
            qdma(lhsT_v[:, c, :], lhsT_in[c])

        load_rnT(0, 0)
        load_lhsT(0)
        load_rnT(0, 1)
        load_lhsT(1)
        load_rnT(0, 2)
        load_lhsT(2)
        load_rnT(0, 3)
        load_lhsT(3)
        qdma(i32[:], ident_in[:])
        qdma(smask[:], smask_in[:])
        qdma(pmask[:], pmask_in[:])
        for g in range(1, NG):
            for c in range(KC):
                load_rnT(g, c)

        dvtabs = persist.tile([128, NM * NCAND], dt.float32, tag="dvtabs")
        rstabs = persist.tile([128, NM * NG], dt.float32, tag="rstabs")
        outbuf = persist.tile([128, 2 * NM], dt.float32, tag="outbuf")

        for g in range(NG):
            for t in range(NM):
                ps = gpsum.tile([128, GW], dt.float32, tag="ps")
                for cp in range(2):
                    for h in range(4):
                        nc.tensor.matmul(
                            ps[:, h * 512:(h + 1) * 512],
                            lhsT_v[:, 2 * cp:2 * cp + 2,
                                   t * 128:(t + 1) * 128],
                            rnT_v[:, 2 * cp:2 * cp + 2,
                                  g * GW + h * 512:g * GW + (h + 1) * 512],
                            perf_mode=mybir.MatmulPerfMode.DoubleRow,
                            start=(cp == 0), stop=(cp == 1))
                # diag candidates: local blocks t and t+8 (self or partner
                # diagonal when 2g+half == c resp. (c+4)%8)
                for half in range(2):
                    o = (t + 8 * half) * 128
                    scr = scrp.tile([128, 128], dt.float32, tag="scr128")
                    col = t * NCAND + 2 * g + half
                    nc.vector.scalar_tensor_tensor(
                        out=scr[:], in0=ps[:, o:o + 128], scalar=1.0,
                        in1=i32[:], op0=Alu.mult, op1=Alu.mult,
                        accum_out=dvtabs[:, col:col + 1])
                e = ep.tile([128, GW], dt.bfloat16, tag="e")
                nc.scalar.activation(
                    e[:], ps[:], Act.Exp, scale=inv_tau,
                    accum_out=rstabs[:, t * NG + g:t * NG + g + 1])

        # ---- epilogue per m-tile ----
        for t in range(NM):
            scr8 = smallp.tile([128, NCAND], dt.float32, tag="scr8")
            selfsim = smallp.tile([128, 1], dt.float32, tag="selfsim")
            nc.vector.scalar_tensor_tensor(
                out=scr8[:], in0=dvtabs[:, t * NCAND:(t + 1) * NCAND],
                scalar=1.0, in1=smask[:], op0=Alu.mult, op1=Alu.mult,
                accum_out=selfsim[:])
            scr8b = smallp.tile([128, NCAND], dt.float32, tag="scr8b")
            nc.vector.scalar_tensor_tensor(
                out=scr8b[:], in0=dvtabs[:, t * NCAND:(t + 1) * NCAND],
                scalar=1.0, in1=pmask[:], op0=Alu.mult, op1=Alu.mult,
                accum_out=outbuf[:, NM + t:NM + t + 1])
            selfexp = smallp.tile([128, 1], dt.float32, tag="selfexp")
            nc.scalar.activation(selfexp[:], selfsim[:], Act.Exp,
                                 scale=inv_tau)
            rowsum = smallp.tile([128, 1], dt.float32, tag="rowsum")
            nc.vector.reduce_sum(rowsum[:], rstabs[:, t * NG:(t + 1) * NG],
                                 axis=mybir.AxisListType.X)
            nc.vector.tensor_sub(outbuf[:, t:t + 1], rowsum[:], selfexp[:])

        nc.sync.dma_start(out[:], outbuf[:])

    # Restrict bacc's activation-table choices to the one table that holds
    # Exp+Copy together, so exactly one ACT table load is emitted.
    import concourse.bacc as bacc_mod
    _orig_tables = bacc_mod.get_activation_tables

    def _only_lnexp(arch):
        keep = "natural_log_exp_and_others"
        return {k: (v if k == keep else set())
                for k, v in _orig_tables(arch).items()}

    bacc_mod.get_activation_tables = _only_lnexp
    try:
        nc.compile()
    finally:
        bacc_mod.get_activation_tables = _orig_tables
    return nc


def _host_inputs(zi, zj):
    reps = np.concatenate([np.asarray(zi, np.float32),
                           np.asarray(zj, np.float32)], axis=0)
    norms = np.maximum(np.sqrt((reps * reps).sum(axis=1, keepdims=True)),
                       1e-8)
    rn8 = (reps / norms).astype(ml_dtypes.float8_e4m3fn)        # [2B, D]
    rnT = np.ascontiguousarray(rn8.T.reshape(KC, 128, ROWS))
    ident_f32 = np.eye(128, dtype=np.float32)
    in_maps = []
    for c in range(NCORES):
        lhsT = np.ascontiguousarray(
            rn8[c * RPC:(c + 1) * RPC].T.reshape(KC, 128, RPC))
        smask = np.zeros((128, NCAND), np.float32)
        smask[:, c] = 1.0
        pmask = np.zeros((128, NCAND), np.float32)
        pmask[:, (c + 4) % 8] = 1.0
        in_maps.append({
            "rnT": rnT, "lhsT": lhsT, "ident_f32": ident_f32,
            "selfmask": smask, "posmask": pmask,
        })
    return in_maps


def _postprocess(results):
    denom = np.empty((ROWS,), np.float64)
    pos = np.empty((ROWS,), np.float64)
    for c in range(NCORES):
        o = np.asarray(results[c]["out"], np.float64)  # [128, 16]
        for t in range(NM):
            rows = slice(c * RPC + t * 128, c * RPC + (t + 1) * 128)
            denom[rows] = o[:, t]
            pos[rows] = o[:, NM + t]
    loss = np.mean(-pos / TAU + np.log(denom))
    return np.asarray(loss, dtype=np.float32)


def kernel(zi, zj, _trace=False):
    from concourse.bass_utils import run_bass_kernel_spmd

    if "nc" not in _prog_cache:
        _prog_cache["nc"] = _build_program()
    nc = _prog_cache["nc"]
    in_maps = _host_inputs(zi, zj)
    res = run_bass_kernel_spmd(nc, in_maps, list(range(NCORES)),
                               trace=_trace)
    _prog_cache["last_result"] = res
    return _postprocess(res.results)


# revision 8
# speedup vs baseline: 2.0273x; 1.0304x over previous
"""NT-Xent / SimCLR contrastive loss on 8 Trainium2 NeuronCores (Bass/Tile).

Problem: zi, zj [4096, 512] f32 -> scalar loss.
  reps = concat(zi, zj)            [8192, 512]
  rn   = reps / max(||reps||, 1e-8)
  sim  = rn @ rn.T                 [8192, 8192]
  pos_i  = sim[i, (i+B) mod 2B]
  denom_i = sum_{j != i} exp(sim_ij / tau)
  loss = mean(-pos/tau + log(denom))

Sharding (per the hint: each device holds its row block of normalized reps
plus the full normalized reps for the GEMM): core c owns sim rows
[c*1024, (c+1)*1024). The host normalizes in f32 (identical math to the
reference) and ships the full normalized reps transposed + quantized to
fp8e4m3 (rnT, replicated) and the core's own row-block slice (lhsT). The
device then runs a pure fp8 DoubleRow GEMM -> Exp -> row-reduce pipeline
with nothing on the critical path ahead of the first matmul.

Numerics:
- Diagonal exclusion is exact: sim_ii is extracted from PSUM (identity-mask
  multiply+reduce, f32), passed through the same ACT Exp LUT, and
  subtracted, so the huge exp(sim_ii/tau) term cancels bit-exactly.
- Positives come from the same PSUM via the partner-block diagonal. For
  m-tile t and 2048-col group g, the diagonal of local 128-col blocks t and
  t+8 covers, across g, all 8 possible positions of both the self and the
  partner diagonals (position 2g+half == c resp. (c+4) mod 8); host-side
  one-hot masks select the right candidate per core.
"""

import sys

for _p in ("/opt/trn_rl_repo",):
    if _p not in sys.path:
        sys.path.insert(0, _p)

from contextlib import ExitStack

import ml_dtypes
import numpy as np

TAU = 0.07
B, D = 4096, 512
NCORES = 8
ROWS = 2 * B              # 8192
RPC = ROWS // NCORES      # 1024 rows per core
NM = RPC // 128           # 8 m-tiles per core
KC = D // 128             # 4 contraction chunks
NG = 4                    # column groups
GW = ROWS // NG           # 2048 cols per group
NCAND = 2 * NG            # 8 diag candidates per m-tile

_prog_cache = {}


def _build_program():
    import concourse.bacc as bacc
    import concourse.tile as tile
    import concourse.mybir as mybir

    dt = mybir.dt
    Alu = mybir.AluOpType
    Act = mybir.ActivationFunctionType

    nc = bacc.Bacc("TRN2", target_bir_lowering=False, debug=False,
                   enable_asserts=False, num_devices=NCORES)

    rnT_in = nc.dram_tensor("rnT", [KC, 128, ROWS], dt.float8e4,
                            kind="ExternalInput").ap()
    lhsT_in = nc.dram_tensor("lhsT", [KC, 128, RPC], dt.float8e4,
                             kind="ExternalInput").ap()
    ident_in = nc.dram_tensor("ident_f32", [128, 128], dt.float32,
                              kind="ExternalInput").ap()
    smask_in = nc.dram_tensor("selfmask", [128, NCAND], dt.float32,
                              kind="ExternalInput").ap()
    pmask_in = nc.dram_tensor("posmask", [128, NCAND], dt.float32,
                              kind="ExternalInput").ap()
    out = nc.dram_tensor("out", [128, 2 * NM], dt.float32,
                         kind="ExternalOutput").ap()

    inv_tau = float(1.0 / TAU)

    with tile.TileContext(nc) as tc, ExitStack() as ctx:
        const = ctx.enter_context(tc.tile_pool(name="const", bufs=1))
        persist = ctx.enter_context(tc.tile_pool(name="persist", bufs=1))
        ep = ctx.enter_context(tc.tile_pool(name="ep", bufs=3))
        scrp = ctx.enter_context(tc.tile_pool(name="scrp", bufs=4))
        smallp = ctx.enter_context(tc.tile_pool(name="smallp", bufs=4))
        gpsum = ctx.enter_context(tc.tile_pool(name="gpsum", bufs=2,
                                               space="PSUM"))

        # Dummy activation issued first so the ACT table load (which bacc
        # attaches to the first activation) happens during the DMA prologue
        # instead of stalling the first real Exp.
        warm = const.tile([128, 1], dt.float32, tag="warm")
        nc.gpsimd.memset(warm[:], 0.0)
        warm2 = const.tile([128, 1], dt.float32, tag="warm2")
        nc.scalar.activation(warm2[:], warm[:], Act.Exp, scale=1.0)

        i32 = const.tile([128, 128], dt.float32, tag="i32")
        smask = const.tile([128, NCAND], dt.float32, tag="smask")
        pmask = const.tile([128, NCAND], dt.float32, tag="pmask")

        lhsT = persist.tile([128, KC * RPC], dt.float8e4, tag="lhsT")
        lhsT_v = lhsT[:].rearrange("p (c w) -> p c w", c=KC)
        rnT = persist.tile([128, KC * ROWS], dt.float8e4, tag="rnT")
        rnT_v = rnT[:].rearrange("p (c w) -> p c w", c=KC)

        # Input DMAs ride the sync + gpsimd queues only (both engines are
        # otherwise idle; issuing from scalar would delay the first Exp since
        # engine instruction streams are in-order). The first GEMM unit's
        # dependencies (rnT group 0 chunks 0-1, lhsT chunks 0-1) are issued
        # first so the PE starts as early as possible.
        qs = [nc.sync, nc.gpsimd]
        nq = 0

        def qdma(dst, src):
            nonlocal nq
            qs[nq % len(qs)].dma_start(dst, src)
            nq += 1

        def load_rnT(g, c):
            qdma(rnT_v[:, c, g * GW:(g + 1) * GW],
                 rnT_in[c, :, g * GW:(g + 1) * GW])

        def load_lhsT(c):
            qdma(lhsT_v[:, c, :], lhsT_in[c])

        load_rnT(0, 0)
        load_lhsT(0)
        load_rnT(0, 1)
        load_lhsT(1)
        load_rnT(0, 2)
        load_lhsT(2)
        load_rnT(0, 3)
        load_lhsT(3)
        qdma(i32[:], ident_in[:])
        qdma(smask[:], smask_in[:])
        qdma(pmask[:], pmask_in[:])
        for g in range(1, NG):
            for c in range(KC):
                load_rnT(g, c)

        dvtabs = persist.tile([128, NM * NCAND], dt.float32, tag="dvtabs")
        rstabs = persist.tile([128, NM * NG], dt.float32, tag="rstabs")
        outbuf = persist.tile([128, 2 * NM], dt.float32, tag="outbuf")

        for g in range(NG):
            for t in range(NM):
                ps = gpsum.tile([128, GW], dt.float32, tag="ps")
                for cp in range(2):
                    for h in range(4):
                        nc.tensor.matmul(
                            ps[:, h * 512:(h + 1) * 512],
                            lhsT_v[:, 2 * cp:2 * cp + 2,
                                   t * 128:(t + 1) * 128],
                            rnT_v[:, 2 * cp:2 * cp + 2,
                                  g * GW + h * 512:g * GW + (h + 1) * 512],
                            perf_mode=mybir.MatmulPerfMode.DoubleRow,
                            start=(cp == 0), stop=(cp == 1))
                # diag candidates: local blocks t and t+8 (self or partner
                # diagonal when 2g+half == c resp. (c+4)%8)
                for half in range(2):
                    o = (t + 8 * half) * 128
                    scr = scrp.tile([128, 128], dt.float32, tag="scr128")
                    col = t * NCAND + 2 * g + half
                    nc.vector.scalar_tensor_tensor(
                        out=scr[:], in0=ps[:, o:o + 128], scalar=1.0,
                        in1=i32[:], op0=Alu.mult, op1=Alu.mult,
                        accum_out=dvtabs[:, col:col + 1])
                e = ep.tile([128, GW], dt.bfloat16, tag="e")
                nc.scalar.activation(
                    e[:], ps[:], Act.Exp, scale=inv_tau,
                    accum_out=rstabs[:, t * NG + g:t * NG + g + 1])
                if g == NG - 1:
                    # epilogue for m-tile t, emitted inline so it overlaps
                    # the remaining units' exps instead of queuing after them
                    scr8 = smallp.tile([128, NCAND], dt.float32, tag="scr8")
                    selfsim = smallp.tile([128, 1], dt.float32,
                                          tag="selfsim")
                    nc.vector.scalar_tensor_tensor(
                        out=scr8[:],
                        in0=dvtabs[:, t * NCAND:(t + 1) * NCAND],
                        scalar=1.0, in1=smask[:], op0=Alu.mult,
                        op1=Alu.mult, accum_out=selfsim[:])
                    scr8b = smallp.tile([128, NCAND], dt.float32,
                                        tag="scr8b")
                    nc.vector.scalar_tensor_tensor(
                        out=scr8b[:],
                        in0=dvtabs[:, t * NCAND:(t + 1) * NCAND],
                        scalar=1.0, in1=pmask[:], op0=Alu.mult,
                        op1=Alu.mult,
                        accum_out=outbuf[:, NM + t:NM + t + 1])
                    selfexp = smallp.tile([128, 1], dt.float32,
                                          tag="selfexp")
                    nc.scalar.activation(selfexp[:], selfsim[:], Act.Exp,
                                         scale=inv_tau)
                    rowsum = smallp.tile([128, 1], dt.float32, tag="rowsum")
                    nc.vector.reduce_sum(
                        rowsum[:], rstabs[:, t * NG:(t + 1) * NG],
                        axis=mybir.AxisListType.X)
                    nc.vector.tensor_sub(outbuf[:, t:t + 1], rowsum[:],
                                         selfexp[:])

        nc.sync.dma_start(out[:], outbuf[:])

    # Restrict bacc's activation-table choices to the one table that holds
    # Exp+Copy together, so exactly one ACT table load is emitted.
    import concourse.bacc as bacc_mod
    _orig_tables = bacc_mod.get_activation_tables

    def _only_lnexp(arch):
        keep = "natural_log_exp_and_others"
        return {k: (v if k == keep else set())
                for k, v in _orig_tables(arch).items()}

    bacc_mod.get_activation_tables = _only_lnexp
    try:
        nc.compile()
    finally:
        bacc_mod.get_activation_tables = _orig_tables
    return nc


def _host_inputs(zi, zj):
    reps = np.concatenate([np.asarray(zi, np.float32),
                           np.asarray(zj, np.float32)], axis=0)
    norms = np.maximum(np.sqrt((reps * reps).sum(axis=1, keepdims=True)),
                       1e-8)
    rn8 = (reps / norms).astype(ml_dtypes.float8_e4m3fn)        # [2B, D]
    rnT = np.ascontiguousarray(rn8.T.reshape(KC, 128, ROWS))
    ident_f32 = np.eye(128, dtype=np.float32)
    in_maps = []
    for c in range(NCORES):
        lhsT = np.ascontiguousarray(
            rn8[c * RPC:(c + 1) * RPC].T.reshape(KC, 128, RPC))
        smask = np.zeros((128, NCAND), np.float32)
        smask[:, c] = 1.0
        pmask = np.zeros((128, NCAND), np.float32)
        pmask[:, (c + 4) % 8] = 1.0
        in_maps.append({
            "rnT": rnT, "lhsT": lhsT, "ident_f32": ident_f32,
            "selfmask": smask, "posmask": pmask,
        })
    return in_maps


def _postprocess(results):
    denom = np.empty((ROWS,), np.float64)
    pos = np.empty((ROWS,), np.float64)
    for c in range(NCORES):
        o = np.asarray(results[c]["out"], np.float64)  # [128, 16]
        for t in range(NM):
            rows = slice(c * RPC + t * 128, c * RPC + (t + 1) * 128)
            denom[rows] = o[:, t]
            pos[rows] = o[:, NM + t]
    loss = np.mean(-pos / TAU + np.log(denom))
    return np.asarray(loss, dtype=np.float32)


def kernel(zi, zj, _trace=False):
    from concourse.bass_utils import run_bass_kernel_spmd

    if "nc" not in _prog_cache:
        _prog_cache["nc"] = _build_program()
    nc = _prog_cache["nc"]
    in_maps = _host_inputs(zi, zj)
    res = run_bass_kernel_spmd(nc, in_maps, list(range(NCORES)),
                               trace=_trace)
    _prog_cache["last_result"] = res
    return _postprocess(res.results)
